# revision 7
# baseline (speedup 1.0000x reference)
"""CWTConvNet Trainium2 kernel — flipped (weight-Toeplitz) formulation.

The reference reduces exactly (see previous baseline) to

    out72[f, s, l] = sum_{j=0}^{351} w2[f, j] * xe[s, j + l],  l in [0, 72)

with w2 = w_real[:, 0, 209:561] and xe = [71 zeros, x[s, 0:352], pad], then an
index-repeat expansion 72 -> 224 (IMG_SELECT) on the host.

Instead of im2col-ing the DATA (2.43 MB of HBM reads per core), this kernel
im2cols the WEIGHTS, which are shared by all 48 signals on a core:

    l = 9*lb + l',  lb in [0,8), l' in [0,9)
    OUT[(s,lb), (f,l')] = sum_c sum_p  xe[s, 128c + p + 9*lb] * w2[f, 128c + p - l']

Per pass of 16 signals, the stationary operand is the (tiny, host-built)
data tile stat[c][p, (s,lb)] = xe[s, 128c+p+9lb] (128x128, full PE array),
and the moving rhs is the weight-Toeplitz wt[c][p, (f,l')] = w2[f, 128c+p-l'],
which is signal-independent. Filter support lets chunks 1/2 carry only
filters 48..111 / 104..111 (and chunk 2 only rows p < 104), so the whole
weight bank is just 1008+576+72 columns. Per-core HBM traffic drops from
~4.1 MB (baseline) to ~1.5 MB: 718 KB of loads + 774 KB of bf16 stores.

A short burst of zero matmuls into a scratch PSUM bank warms the PE HAM
clock gate while the loads stream, so the real matmuls run at 2.4 GHz.
"""

import numpy as np

import concourse.bacc as bacc
import concourse.bass as bass
import concourse.mybir as mybir
import concourse.tile as tile
from concourse.bass_utils import run_bass_kernel_spmd

# Problem constants (hardcoded; kernel.py must be self-contained).
B, C, L = 32, 12, 2048
F, K = 112, 561
KOFF = 209                 # first needed tap; w2 = w_real[:, 0, 209:561]
J = 352                    # taps per filter window
NCORES = 8
BPC = B // NCORES          # batches per core
S = BPC * C                # signals per core (48)
NL = 72                    # conv output positions actually used
NI = 224                   # expanded output length

NT = 3                     # passes per core (16 signals each)
SG = 16                    # signals per pass
NLB, NLP = 8, 9            # l = NLP*lb + l'; SG*NLB = 128 partitions exactly
F1LO, F2LO = 48, 104       # first filter with support in chunks 1 / 2
NC0 = F * NLP              # 1008 weight cols, chunk 0
NC1 = (F - F1LO) * NLP     # 576,  chunk 1 (filters 48..111)
NC2 = (F - F2LO) * NLP     # 72,   chunk 2 (filters 104..111)
C2ROWS = 104               # chunk 2 rows p >= 104 are all-zero weights
XE_LEN = 448               # 71 zeros + 352 signal + tail (max index 446)
XE_ZLEAD = 71

NDUMMY = 14                # PE warm-up matmuls (zeros) while loads stream

SEL = np.linspace(0, 71, NI, dtype=np.int64)

_CACHE = {}


def _build_nc():
    f32 = mybir.dt.float32
    bf16 = mybir.dt.bfloat16
    nc = bacc.Bacc("TRN2", target_bir_lowering=False, debug=False)

    # packA: stat(0,0) | wt0[:, 0:504]  -> pass-0 chunk-0 bank-A matmul early
    # packA2: wt0[:, 504:1008]
    # packB: stat(0,1) | stat(0,2) | wt1 | wt2
    # packC: stat(1,0..2) | stat(2,0..2)
    packA_d = nc.declare_dram_parameter("packA", [128, 128 + 504], bf16, isOutput=False)
    packA2_d = nc.declare_dram_parameter("packA2", [128, 504], bf16, isOutput=False)
    packB_d = nc.declare_dram_parameter("packB", [128, 256 + NC1 + NC2], bf16, isOutput=False)
    packC_d = nc.declare_dram_parameter("packC", [128, 6 * 128], bf16, isOutput=False)
    # y[t, bank, :, :]: bank 0 = filters 0..55, bank 1 = filters 56..111
    y_d = nc.declare_dram_parameter("y", [NT, 2, 128, 504], bf16, isOutput=True)

    with tile.TileContext(nc) as tc:
        with (
            tc.tile_pool(name="sbuf", bufs=1) as pool,
            tc.tile_pool(name="psum", bufs=1, space="PSUM") as psum_pool,
        ):
            # Loads first so the sync engine issues them the moment its
            # preamble ends.
            bigA = pool.tile([128, 128 + 504], bf16, tag="bigA", name="bigA")
            bigA2 = pool.tile([128, 504], bf16, tag="bigA2", name="bigA2")
            bigB = pool.tile([128, 256 + NC1 + NC2], bf16, tag="bigB", name="bigB")
            bigC = pool.tile([128, 6 * 128], bf16, tag="bigC", name="bigC")
            nc.sync.dma_start(out=bigA[:], in_=packA_d.ap())
            nc.sync.dma_start(out=bigA2[:], in_=packA2_d.ap())
            nc.sync.dma_start(out=bigB[:], in_=packB_d.ap())
            nc.sync.dma_start(out=bigC[:], in_=packC_d.ap())

            # PE HAM warm-up: zero matmuls into a scratch bank while the
            # loads stream. Never read back.
            scratch = pool.tile([128, 128], bf16, tag="scr", name="scr")
            nc.vector.memset(scratch[:], 0)
            ps_dummy = psum_pool.tile([128, 504], f32, tag="psD", name="psD")
            for _ in range(NDUMMY):
                nc.tensor.matmul(
                    ps_dummy[:, 0:128], scratch[:, :], scratch[:, :],
                    start=True, stop=True,
                )

            wt0a = bigA[:, 128 : 128 + 504]
            wt0b = bigA2[:, :]
            wt1 = bigB[:, 256 : 256 + NC1]
            wt2 = bigB[:, 256 + NC1 : 256 + NC1 + NC2]
            stat = {
                (0, 0): bigA[:, 0:128],
                (0, 1): bigB[:, 0:128],
                (0, 2): bigB[:, 128:256],
            }
            for t in (1, 2):
                for c in range(3):
                    off = ((t - 1) * 3 + c) * 128
                    stat[(t, c)] = bigC[:, off : off + 128]

            for t in range(NT):
                psA = psum_pool.tile([128, 504], f32, tag=f"psA{t}", name=f"psA{t}")
                psB = psum_pool.tile([128, 504], f32, tag=f"psB{t}", name=f"psB{t}")
                st0, st1, st2 = stat[(t, 0)], stat[(t, 1)], stat[(t, 2)]
                # chunk 0 (all filters), stationary st0
                nc.tensor.matmul(psA[:, :], st0, wt0a, start=True, stop=False)
                nc.tensor.matmul(psB[:, :], st0, wt0b, start=True, stop=False)
                # chunk 1 (filters 48..111), stationary st1
                nc.tensor.matmul(psA[:, 432:504], st1, wt1[:, 0:72], start=False, stop=True)
                nc.tensor.matmul(psB[:, :], st1, wt1[:, 72:NC1], start=False, stop=False)
                # chunk 2 (filters 104..111, rows < 104), stationary st2
                nc.tensor.matmul(
                    psB[:, 432:504], st2[0:C2ROWS, :], wt2[0:C2ROWS, :],
                    start=False, stop=True,
                )
                # Drain both banks (fp32 -> bf16) on the two PSUM-capable
                # engines in parallel; store each half as soon as its drain
                # lands. Stores share the sync HWDGE ring (idle after the
                # loads), keeping the scalar engine free to drain.
                o = pool.tile([128, NC0], bf16, tag=f"o{t}", name=f"o{t}")
                nc.scalar.copy(o[:, 0:504], psA[:, :])
                nc.vector.tensor_copy(out=o[:, 504:NC0], in_=psB[:, :])
                nc.sync.dma_start(out=y_d.ap()[t, 0], in_=o[:, 0:504])
                nc.sync.dma_start(out=y_d.ap()[t, 1], in_=o[:, 504:NC0])

    nc.compile()
    return nc


def _get_nc():
    if "nc" not in _CACHE:
        _CACHE["nc"] = _build_nc()
    return _CACHE["nc"]


def _build_wt(w2pad):
    """wt[c][p, (f-flo)*9+l'] = w2[f, 128c + p - l'] (zero outside [0, J))."""
    out = []
    for c, flo, rows in ((0, 0, 128), (1, F1LO, 128), (2, F2LO, C2ROWS)):
        p = np.arange(128)[:, None, None]
        f = np.arange(flo, F)[None, :, None]
        lp = np.arange(NLP)[None, None, :]
        j = 128 * c + p - lp
        val = np.where((j >= 0) & (j < J), w2pad[f, np.clip(j, 0, J - 1)], 0.0)
        val[rows:] = 0.0
        out.append(val.reshape(128, (F - flo) * NLP).astype(np.float32))
    return out


def _prepare_in_maps(x, w_real):
    import ml_dtypes

    np_bf16 = np.dtype(ml_dtypes.bfloat16)
    x = np.ascontiguousarray(np.asarray(x), dtype=np.float32)
    w_real = np.asarray(w_real, dtype=np.float32)

    w2 = w_real[:, 0, KOFF:K]                       # [112, 352]
    wt0, wt1, wt2 = _build_wt(w2)

    # Stationary index grid: q[c][p, sl, lb] = 128c + p + 9lb
    p = np.arange(128)[:, None, None]
    lb = np.arange(NLB)[None, None, :]
    qs = [128 * c + p + NLP * lb for c in range(3)]  # each [128, 1, 8]

    in_maps = []
    for m in range(NCORES):
        xc = x[m * BPC : (m + 1) * BPC].reshape(S, L)
        xe = np.zeros((S, XE_LEN), np.float32)
        xe[:, XE_ZLEAD : XE_ZLEAD + J] = xc[:, :J]
        st = {}
        for t in range(NT):
            sig = xe[SG * t : SG * (t + 1)]          # [16, 448]
            for c in range(3):
                # [128, 16, 8] -> [128, 128] with col = sl*8 + lb
                v = sig[np.arange(SG)[None, :, None], qs[c]]
                st[(t, c)] = v.reshape(128, SG * NLB)
        packA = np.concatenate([st[(0, 0)], wt0[:, 0:504]], axis=1)
        packA2 = wt0[:, 504:NC0]
        packB = np.concatenate([st[(0, 1)], st[(0, 2)], wt1, wt2], axis=1)
        packC = np.concatenate(
            [st[(t, c)] for t in (1, 2) for c in range(3)], axis=1
        )
        in_maps.append({
            "packA": np.ascontiguousarray(packA).astype(np_bf16),
            "packA2": np.ascontiguousarray(packA2).astype(np_bf16),
            "packB": np.ascontiguousarray(packB).astype(np_bf16),
            "packC": np.ascontiguousarray(packC).astype(np_bf16),
        })
    return in_maps


def _assemble(results):
    # Device output y[t, bank, sl*8+lb, (f-56*bank)*9+l'] = out72[f, 16t+sl, 9lb+l'].
    ydev = np.stack([np.asarray(r["y"], dtype=np.float32) for r in results])
    # [8, NT, 2, 128, 504] -> [8, NT, 128, 1008] (bank-major filter split)
    ydev = ydev.transpose(0, 1, 3, 2, 4).reshape(NCORES, NT, 128, NC0)
    yv = ydev.reshape(NCORES, NT, SG, NLB, F, NLP)
    out72 = yv.transpose(0, 1, 2, 4, 3, 5).reshape(NCORES, S, F, NL)
    y = out72[..., SEL]                              # [8, S, F, NI]
    return np.ascontiguousarray(y.reshape(B, C, F, NI), dtype=np.float32)


def kernel(x, w_real):
    nc = _get_nc()
    in_maps = _prepare_in_maps(x, w_real)
    res = run_bass_kernel_spmd(nc, in_maps, list(range(NCORES)))
    return _assemble(res.results)


# revision 14
# speedup vs baseline: 1.1487x; 1.1487x over previous
"""CWTConvNet Trainium2 kernel — flipped (weight-Toeplitz) formulation.

The reference reduces exactly (see previous baseline) to

    out72[f, s, l] = sum_{j=0}^{351} w2[f, j] * xe[s, j + l],  l in [0, 72)

with w2 = w_real[:, 0, 209:561] and xe = [71 zeros, x[s, 0:352], pad], then an
index-repeat expansion 72 -> 224 (IMG_SELECT) on the host.

Instead of im2col-ing the DATA (2.43 MB of HBM reads per core), this kernel
im2cols the WEIGHTS, which are shared by all 48 signals on a core:

    l = 9*lb + l',  lb in [0,8), l' in [0,9)
    OUT[(s,lb), (f,l')] = sum_c sum_p  xe[s, 128c + p + 9*lb] * w2[f, 128c + p - l']

Per pass of 16 signals, the stationary operand is the (tiny, host-built)
data tile stat[c][p, (s,lb)] = xe[s, 128c+p+9lb] (128x128, full PE array),
and the moving rhs is the weight-Toeplitz wt[c][p, (f,l')] = w2[f, 128c+p-l'],
which is signal-independent. Filter support lets chunks 1/2 carry only
filters 48..111 / 104..111 (and chunk 2 only rows p < 104), so the whole
weight bank is just 1008+576+72 columns. Per-core HBM traffic drops from
~4.1 MB (baseline) to ~1.5 MB: 718 KB of loads + 774 KB of bf16 stores.

A short burst of zero matmuls into a scratch PSUM bank warms the PE HAM
clock gate while the loads stream, so the real matmuls run at 2.4 GHz.
"""

import numpy as np

import concourse.bacc as bacc
import concourse.bass as bass
import concourse.mybir as mybir
import concourse.tile as tile
from concourse.bass_utils import run_bass_kernel_spmd

# Problem constants (hardcoded; kernel.py must be self-contained).
B, C, L = 32, 12, 2048
F, K = 112, 561
KOFF = 209                 # first needed tap; w2 = w_real[:, 0, 209:561]
J = 352                    # taps per filter window
NCORES = 8
BPC = B // NCORES          # batches per core
S = BPC * C                # signals per core (48)
NL = 72                    # conv output positions actually used
NI = 224                   # expanded output length

NT = 3                     # passes per core (16 signals each)
SG = 16                    # signals per pass
NLB, NLP = 8, 9            # l = NLP*lb + l'; SG*NLB = 128 partitions exactly
F1LO, F2LO = 48, 104       # first filter with support in chunks 1 / 2
NC0 = F * NLP              # 1008 weight cols, chunk 0
NC1 = (F - F1LO) * NLP     # 576,  chunk 1 (filters 48..111)
NC2 = (F - F2LO) * NLP     # 72,   chunk 2 (filters 104..111)
C2ROWS = 104               # chunk 2 rows p >= 104 are all-zero weights
XE_LEN = 448               # 71 zeros + 352 signal + tail (max index 446)
XE_ZLEAD = 71

NDUMMY = 6                 # PE warm-up matmuls while loads stream

SEL = np.linspace(0, 71, NI, dtype=np.int64)

_CACHE = {}


def _build_nc():
    f32 = mybir.dt.float32
    bf16 = mybir.dt.bfloat16
    nc = bacc.Bacc("TRN2", target_bir_lowering=False, debug=False)

    # packA: stat(0,0) | wt0[:, 0:504]  -> pass-0 chunk-0 bank-A matmul early
    # packA2: wt0[:, 504:1008]
    # packB: stat(0,1) | stat(0,2) | wt1 | wt2
    # packC: stat(1,0..2) | stat(2,0..2)
    packA_d = nc.declare_dram_parameter("packA", [128, 128 + 504], bf16, isOutput=False)
    packA2_d = nc.declare_dram_parameter("packA2", [128, 504], bf16, isOutput=False)
    packB_d = nc.declare_dram_parameter("packB", [128, 256 + NC1 + NC2], bf16, isOutput=False)
    packC_d = nc.declare_dram_parameter("packC", [128, 6 * 128], bf16, isOutput=False)
    y_d = nc.declare_dram_parameter("y", [NT, 128, NC0], bf16, isOutput=True)

    with tile.TileContext(nc) as tc:
        with (
            tc.tile_pool(name="sbuf", bufs=1) as pool,
            tc.tile_pool(name="psum", bufs=1, space="PSUM") as psum_pool,
        ):
            # Warm-up scratch memset first so the vector engine clears it
            # the moment its preamble ends (the dummy matmuls chase it).
            scratch = pool.tile([128, 504], bf16, tag="scr", name="scr")
            nc.vector.memset(scratch[:], 0)

            # Loads next so the sync engine issues them the moment its
            # preamble ends.
            bigA = pool.tile([128, 128 + 504], bf16, tag="bigA", name="bigA")
            bigA2 = pool.tile([128, 504], bf16, tag="bigA2", name="bigA2")
            bigB = pool.tile([128, 256 + NC1 + NC2], bf16, tag="bigB", name="bigB")
            bigC = pool.tile([128, 6 * 128], bf16, tag="bigC", name="bigC")
            nc.sync.dma_start(out=bigA[:], in_=packA_d.ap())
            nc.sync.dma_start(out=bigA2[:], in_=packA2_d.ap())
            nc.sync.dma_start(out=bigB[:], in_=packB_d.ap())
            nc.sync.dma_start(out=bigC[:], in_=packC_d.ap())

            # PE HAM warm-up: back-to-back zero matmuls into a scratch bank
            # while the loads stream — ~2.5us of continuous PE activity so
            # the clock gate opens before the real matmuls start.
            ps_dummy = psum_pool.tile([128, 504], f32, tag="psD", name="psD")
            for _ in range(NDUMMY):
                nc.tensor.matmul(
                    ps_dummy[:, :], scratch[:, 0:128], scratch[:, :],
                    start=True, stop=True,
                )

            wt0a = bigA[:, 128 : 128 + 504]
            wt0b = bigA2[:, :]
            wt1 = bigB[:, 256 : 256 + NC1]
            wt2 = bigB[:, 256 + NC1 : 256 + NC1 + NC2]
            stat = {
                (0, 0): bigA[:, 0:128],
                (0, 1): bigB[:, 0:128],
                (0, 2): bigB[:, 128:256],
            }
            for t in (1, 2):
                for c in range(3):
                    off = ((t - 1) * 3 + c) * 128
                    stat[(t, c)] = bigC[:, off : off + 128]

            for t in range(NT):
                psA = psum_pool.tile([128, 504], f32, tag=f"psA{t}", name=f"psA{t}")
                psB = psum_pool.tile([128, 504], f32, tag=f"psB{t}", name=f"psB{t}")
                st0, st1, st2 = stat[(t, 0)], stat[(t, 1)], stat[(t, 2)]
                # chunk 0 (all filters), stationary st0
                nc.tensor.matmul(psA[:, :], st0, wt0a, start=True, stop=False)
                nc.tensor.matmul(psB[:, :], st0, wt0b, start=True, stop=False)
                # chunk 1 (filters 48..111), stationary st1
                nc.tensor.matmul(psA[:, 432:504], st1, wt1[:, 0:72], start=False, stop=True)
                nc.tensor.matmul(psB[:, :], st1, wt1[:, 72:NC1], start=False, stop=False)
                # chunk 2 (filters 104..111, rows < 104), stationary st2
                nc.tensor.matmul(
                    psB[:, 432:504], st2[0:C2ROWS, :], wt2[0:C2ROWS, :],
                    start=False, stop=True,
                )
                # Drain both banks (fp32 -> bf16) on the two PSUM-capable
                # engines in parallel. Bank A's accumulation ends at the
                # chunk-1 matmul, so its drain overlaps the remaining
                # matmuls. Passes 0/1 store whole; the final pass stores
                # per-bank on separate HWDGE rings to shorten the tail.
                o = pool.tile([128, NC0], bf16, tag=f"o{t}", name=f"o{t}")
                nc.scalar.copy(o[:, 0:504], psA[:, :])
                nc.vector.tensor_copy(out=o[:, 504:NC0], in_=psB[:, :])
                if t < NT - 1:
                    nc.sync.dma_start(out=y_d.ap()[t], in_=o[:])
                else:
                    nc.scalar.dma_start(
                        out=y_d.ap()[t][:, 0:504], in_=o[:, 0:504]
                    )
                    nc.sync.dma_start(
                        out=y_d.ap()[t][:, 504:NC0], in_=o[:, 504:NC0]
                    )

    nc.compile()
    return nc


def _get_nc():
    if "nc" not in _CACHE:
        _CACHE["nc"] = _build_nc()
    return _CACHE["nc"]


def _build_wt(w2pad):
    """wt[c][p, (f-flo)*9+l'] = w2[f, 128c + p - l'] (zero outside [0, J))."""
    out = []
    for c, flo, rows in ((0, 0, 128), (1, F1LO, 128), (2, F2LO, C2ROWS)):
        p = np.arange(128)[:, None, None]
        f = np.arange(flo, F)[None, :, None]
        lp = np.arange(NLP)[None, None, :]
        j = 128 * c + p - lp
        val = np.where((j >= 0) & (j < J), w2pad[f, np.clip(j, 0, J - 1)], 0.0)
        val[rows:] = 0.0
        out.append(val.reshape(128, (F - flo) * NLP).astype(np.float32))
    return out


def _prepare_in_maps(x, w_real):
    import ml_dtypes

    np_bf16 = np.dtype(ml_dtypes.bfloat16)
    x = np.ascontiguousarray(np.asarray(x), dtype=np.float32)
    w_real = np.asarray(w_real, dtype=np.float32)

    w2 = w_real[:, 0, KOFF:K]                       # [112, 352]
    wt0, wt1, wt2 = _build_wt(w2)

    # Stationary index grid: q[c][p, sl, lb] = 128c + p + 9lb
    p = np.arange(128)[:, None, None]
    lb = np.arange(NLB)[None, None, :]
    qs = [128 * c + p + NLP * lb for c in range(3)]  # each [128, 1, 8]

    in_maps = []
    for m in range(NCORES):
        xc = x[m * BPC : (m + 1) * BPC].reshape(S, L)
        xe = np.zeros((S, XE_LEN), np.float32)
        xe[:, XE_ZLEAD : XE_ZLEAD + J] = xc[:, :J]
        st = {}
        for t in range(NT):
            sig = xe[SG * t : SG * (t + 1)]          # [16, 448]
            for c in range(3):
                # [128, 16, 8] -> [128, 128] with col = sl*8 + lb
                v = sig[np.arange(SG)[None, :, None], qs[c]]
                st[(t, c)] = v.reshape(128, SG * NLB)
        packA = np.concatenate([st[(0, 0)], wt0[:, 0:504]], axis=1)
        packA2 = wt0[:, 504:NC0]
        packB = np.concatenate([st[(0, 1)], st[(0, 2)], wt1, wt2], axis=1)
        packC = np.concatenate(
            [st[(t, c)] for t in (1, 2) for c in range(3)], axis=1
        )
        in_maps.append({
            "packA": np.ascontiguousarray(packA).astype(np_bf16),
            "packA2": np.ascontiguousarray(packA2).astype(np_bf16),
            "packB": np.ascontiguousarray(packB).astype(np_bf16),
            "packC": np.ascontiguousarray(packC).astype(np_bf16),
        })
    return in_maps


def _assemble(results):
    # Device output y[t, sl*8+lb, f*9+l'] = out72[f, 16t+sl, 9lb+l'].
    ydev = np.stack([np.asarray(r["y"], dtype=np.float32) for r in results])
    yv = ydev.reshape(NCORES, NT, SG, NLB, F, NLP)
    out72 = yv.transpose(0, 1, 2, 4, 3, 5).reshape(NCORES, S, F, NL)
    y = out72[..., SEL]                              # [8, S, F, NI]
    return np.ascontiguousarray(y.reshape(B, C, F, NI), dtype=np.float32)


def kernel(x, w_real):
    nc = _get_nc()
    in_maps = _prepare_in_maps(x, w_real)
    res = run_bass_kernel_spmd(nc, in_maps, list(range(NCORES)))
    return _assemble(res.results)


# revision 15
# speedup vs baseline: 1.1585x; 1.0086x over previous
"""CWTConvNet Trainium2 kernel — flipped (weight-Toeplitz) formulation.

The reference reduces exactly (see earlier baseline) to

    out72[f, s, l] = sum_{j=0}^{351} w2[f, j] * xe[s, j + l],  l in [0, 72)

with w2 = w_real[:, 0, 209:561] and xe = [71 zeros, x[s, 0:352], pad], then an
index-repeat expansion 72 -> 224 (IMG_SELECT) on the host.

Instead of im2col-ing the DATA (2.43 MB of HBM reads per core), this kernel
im2cols the WEIGHTS, which are shared by all 48 signals on a core:

    l = 9*lb + l',  lb in [0,8), l' in [0,9)
    OUT[(s,lb), (f,l')] = sum_c sum_p  xe[s, 128c + p + 9*lb] * w2[f, 128c + p - l']

Per pass of 16 signals the stationary operand is the (tiny, host-built) data
tile stat[c][p, (s,lb)] = xe[s, 128c+p+9lb] (128x128 — full PE array), and
the moving rhs is the weight-Toeplitz wt[c][p, (f,l')] = w2[f, 128c+p-l'],
which is signal-independent. Filter support limits chunk 1 to filters
48..111 and chunk 2 to filters 104..111 with rows p < 104, so the weight
bank is only 1008+576+72 columns. Per-core HBM traffic drops from ~4.1 MB
(baseline) to ~1.5 MB: ~720 KB of loads + 774 KB of bf16 stores.

Scheduling structure:
  - Load 1 carries everything pass 0 needs, so pass-0 matmuls, drains and
    store start ~1 us before the pass-1/2 stationaries (loads 2/3) finish.
  - The PSUM filter->bank split is PERMUTED (bank A = filters 0..47 +
    104..111, bank B = 48..103) so bank B's accumulation ends one matmul
    early and its drain overlaps the tiny chunk-2 matmul.
  - A burst of zero matmuls into a scratch bank keeps the PE busy from
    preamble end, opening the HAM clock gate before the real matmuls.
  - The final pass stores per-bank on both HWDGE rings to shorten the tail.
"""

import numpy as np

import concourse.bacc as bacc
import concourse.bass as bass
import concourse.mybir as mybir
import concourse.tile as tile
from concourse.bass_utils import run_bass_kernel_spmd

# Problem constants (hardcoded; kernel.py must be self-contained).
B, C, L = 32, 12, 2048
F, K = 112, 561
KOFF = 209                 # first needed tap; w2 = w_real[:, 0, 209:561]
J = 352                    # taps per filter window
NCORES = 8
BPC = B // NCORES          # batches per core
S = BPC * C                # signals per core (48)
NL = 72                    # conv output positions actually used
NI = 224                   # expanded output length

NT = 3                     # passes per core (16 signals each)
SG = 16                    # signals per pass
NLB, NLP = 8, 9            # l = NLP*lb + l'; SG*NLB = 128 partitions exactly
F1LO, F2LO = 48, 104       # first filter with support in chunks 1 / 2
C2ROWS = 104               # chunk-2 rows p >= 104 are all-zero weights
XE_LEN = 448               # 71 zeros + 352 signal + tail (max index 446)
XE_ZLEAD = 71
NDUMMY = 7                 # PE warm-up matmuls while loads stream

# PSUM bank permutation: bank A = filters 0..47 + 104..111, bank B = 48..103.
PERM = np.r_[np.arange(0, 48), np.arange(104, 112), np.arange(48, 104)]
NBK = 56 * NLP             # 504 columns per bank

# pack1 column layout (everything pass 0 needs):
#   stat00 | wt0a | wt0b | stat01 | stat02 | wt1a | wt1b | wt2
P1_COLS = 128 + 504 + 504 + 128 + 128 + 72 + 504 + 72   # 2040

SEL = np.linspace(0, 71, NI, dtype=np.int64)

_CACHE = {}


def _build_nc():
    f32 = mybir.dt.float32
    bf16 = mybir.dt.bfloat16
    nc = bacc.Bacc("TRN2", target_bir_lowering=False, debug=False)

    pack1_d = nc.declare_dram_parameter("pack1", [128, P1_COLS], bf16, isOutput=False)
    pack2_d = nc.declare_dram_parameter("pack2", [128, 384], bf16, isOutput=False)
    pack3_d = nc.declare_dram_parameter("pack3", [128, 384], bf16, isOutput=False)
    y_d = nc.declare_dram_parameter("y", [NT, 128, 2 * NBK], bf16, isOutput=True)

    with tile.TileContext(nc) as tc:
        with (
            tc.tile_pool(name="sbuf", bufs=1) as pool,
            tc.tile_pool(name="psum", bufs=1, space="PSUM") as psum_pool,
        ):
            # Warm-up scratch memset first so the vector engine clears it
            # the moment its preamble ends (the dummy matmuls chase it).
            scratch = pool.tile([128, 504], bf16, tag="scr", name="scr")
            nc.vector.memset(scratch[:], 0)

            # Loads next so the sync engine issues them back to back.
            big1 = pool.tile([128, P1_COLS], bf16, tag="big1", name="big1")
            big2 = pool.tile([128, 384], bf16, tag="big2", name="big2")
            big3 = pool.tile([128, 384], bf16, tag="big3", name="big3")
            nc.sync.dma_start(out=big1[:], in_=pack1_d.ap())
            nc.sync.dma_start(out=big2[:], in_=pack2_d.ap())
            nc.sync.dma_start(out=big3[:], in_=pack3_d.ap())

            # PE HAM warm-up: back-to-back zero matmuls into a scratch bank
            # while the loads stream — continuous PE activity from preamble
            # end opens the clock gate before the real matmuls start.
            ps_dummy = psum_pool.tile([128, 504], f32, tag="psD", name="psD")
            for _ in range(NDUMMY):
                nc.tensor.matmul(
                    ps_dummy[:, :], scratch[:, 0:128], scratch[:, :],
                    start=True, stop=True,
                )

            o = 128
            wt0a = big1[:, o : o + 504]; o += 504
            wt0b = big1[:, o : o + 504]; o += 504
            st01 = big1[:, o : o + 128]; o += 128
            st02 = big1[:, o : o + 128]; o += 128
            wt1a = big1[:, o : o + 72]; o += 72
            wt1b = big1[:, o : o + 504]; o += 504
            wt2 = big1[:, o : o + 72]
            stat = {
                (0, 0): big1[:, 0:128], (0, 1): st01, (0, 2): st02,
                (1, 0): big2[:, 0:128], (1, 1): big2[:, 128:256], (1, 2): big2[:, 256:384],
                (2, 0): big3[:, 0:128], (2, 1): big3[:, 128:256], (2, 2): big3[:, 256:384],
            }

            for t in range(NT):
                psA = psum_pool.tile([128, NBK], f32, tag=f"psA{t}", name=f"psA{t}")
                psB = psum_pool.tile([128, NBK], f32, tag=f"psB{t}", name=f"psB{t}")
                stA, stB, stC = stat[(t, 0)], stat[(t, 1)], stat[(t, 2)]
                # chunk 0 (all filters), stationary stA
                nc.tensor.matmul(psA[:, :], stA, wt0a, start=True, stop=False)
                nc.tensor.matmul(psB[:, :], stA, wt0b, start=True, stop=False)
                # chunk 1: bank-A slice is filters 104..111, bank B 48..103
                nc.tensor.matmul(psA[:, 432:504], stB, wt1a, start=False, stop=False)
                nc.tensor.matmul(psB[:, :], stB, wt1b, start=False, stop=True)
                # chunk 2 (filters 104..111, rows < 104) finishes bank A
                nc.tensor.matmul(
                    psA[:, 432:504], stC[0:C2ROWS, :], wt2[0:C2ROWS, :],
                    start=False, stop=True,
                )
                # Bank B completes first: drain it on the vector engine
                # under the chunk-2 matmul; bank A drains on scalar right
                # after. Final pass stores per-bank on both HWDGE rings.
                ot = pool.tile([128, 2 * NBK], bf16, tag=f"o{t}", name=f"o{t}")
                nc.vector.tensor_copy(out=ot[:, NBK : 2 * NBK], in_=psB[:, :])
                nc.scalar.copy(ot[:, 0:NBK], psA[:, :])
                if t < NT - 1:
                    nc.sync.dma_start(out=y_d.ap()[t], in_=ot[:])
                else:
                    nc.sync.dma_start(
                        out=y_d.ap()[t][:, NBK : 2 * NBK], in_=ot[:, NBK : 2 * NBK]
                    )
                    nc.scalar.dma_start(
                        out=y_d.ap()[t][:, 0:NBK], in_=ot[:, 0:NBK]
                    )

    nc.compile()
    return nc


def _get_nc():
    if "nc" not in _CACHE:
        _CACHE["nc"] = _build_nc()
    return _CACHE["nc"]


def _build_wt(w2):
    """Weight-Toeplitz chunks, permuted into the bank order.

    wt_c[p, cols] with cols enumerating (filter, l') pairs; filter order is
    PERM for chunk 0, [104..111, 48..103] for chunk 1, [104..111] for 2.
    """
    def toep(c, fsel, rows):
        p = np.arange(128)[:, None, None]
        f = np.asarray(fsel)[None, :, None]
        lp = np.arange(NLP)[None, None, :]
        j = 128 * c + p - lp
        val = np.where((j >= 0) & (j < J), w2[f, np.clip(j, 0, J - 1)], 0.0)
        val[rows:] = 0.0
        return val.reshape(128, len(fsel) * NLP).astype(np.float32)

    wt0 = toep(0, PERM, 128)                               # [128, 1008]
    wt1 = toep(1, np.r_[np.arange(104, 112), np.arange(48, 104)], 128)
    wt2 = toep(2, np.arange(104, 112), C2ROWS)
    return wt0[:, 0:504], wt0[:, 504:1008], wt1[:, 0:72], wt1[:, 72:576], wt2


def _prepare_in_maps(x, w_real):
    import ml_dtypes

    np_bf16 = np.dtype(ml_dtypes.bfloat16)
    x = np.ascontiguousarray(np.asarray(x), dtype=np.float32)
    w_real = np.asarray(w_real, dtype=np.float32)

    w2 = w_real[:, 0, KOFF:K]                              # [112, 352]
    wt0a, wt0b, wt1a, wt1b, wt2 = _build_wt(w2)

    # Stationary index grid: q[c][p, sl, lb] = 128c + p + 9lb
    p = np.arange(128)[:, None, None]
    lb = np.arange(NLB)[None, None, :]
    qs = [128 * c + p + NLP * lb for c in range(3)]        # each [128, 1, 8]

    in_maps = []
    for m in range(NCORES):
        xc = x[m * BPC : (m + 1) * BPC].reshape(S, L)
        xe = np.zeros((S, XE_LEN), np.float32)
        xe[:, XE_ZLEAD : XE_ZLEAD + J] = xc[:, :J]
        st = {}
        for t in range(NT):
            sig = xe[SG * t : SG * (t + 1)]                # [16, 448]
            for c in range(3):
                v = sig[np.arange(SG)[None, :, None], qs[c]]
                st[(t, c)] = v.reshape(128, SG * NLB)      # col = sl*8 + lb
        pack1 = np.concatenate(
            [st[(0, 0)], wt0a, wt0b, st[(0, 1)], st[(0, 2)], wt1a, wt1b, wt2],
            axis=1,
        )
        pack2 = np.concatenate([st[(1, c)] for c in range(3)], axis=1)
        pack3 = np.concatenate([st[(2, c)] for c in range(3)], axis=1)
        in_maps.append({
            "pack1": np.ascontiguousarray(pack1).astype(np_bf16),
            "pack2": np.ascontiguousarray(pack2).astype(np_bf16),
            "pack3": np.ascontiguousarray(pack3).astype(np_bf16),
        })
    return in_maps


def _assemble(results):
    # Device output y[t, sl*8+lb, 9*i+l'] = out72[PERM[i], 16t+sl, 9lb+l'].
    ydev = np.stack([np.asarray(r["y"], dtype=np.float32) for r in results])
    yv = ydev.reshape(NCORES, NT, SG, NLB, F, NLP)
    o = yv.transpose(0, 1, 2, 4, 3, 5).reshape(NCORES, S, F, NL)
    out72 = np.empty_like(o)
    out72[:, :, PERM, :] = o                               # undo bank permutation
    y = out72[..., SEL]                                    # [8, S, F, NI]
    return np.ascontiguousarray(y.reshape(B, C, F, NI), dtype=np.float32)


def kernel(x, w_real):
    nc = _get_nc()
    in_maps = _prepare_in_maps(x, w_real)
    res = run_bass_kernel_spmd(nc, in_maps, list(range(NCORES)))
    return _assemble(res.results)
